# revision 8
# baseline (speedup 1.0000x reference)
# Trainium2 Bass kernel for nn_NegativeSamplingBCELoss.
#
# Reference computation (per batch row b of B=8192, classes C=2048):
#   pos = targets, neg = 1-targets, num_pos = sum(pos)
#   k = floor(max(num_pos,1) * 5)
#   avg_sim = (pos @ similarity) / max(num_pos, 1)
#   w = (1 - avg_sim) * neg
#   scores = log(max(w,1e-30)) + gumbel(key=42)  (for w>0, else -inf)
#   select top-k_eff scores per row (k_eff = min(k, #finite))
#   final_mask = pos + selected
#   loss = sum(bce(logits,targets)*final_mask) / sum(final_mask)
#
# Device strategy (8 cores, batch-sharded 1024 rows each):
#   - PE: psum = pos @ (bf16(similarity) + 4096*I).  The poisoned diagonal
#     adds 4096 to psum exactly at positive positions, forcing their
#     "1-avg_sim" argument negative so they can never be selected.  The true
#     diagonal never contributes to scores at negatives (c' ranges over
#     positives != c).
#   - ACT: y = Relu(psum*(-1/np) + (1-2^-23)); logw = Ln(y + 2^-23)
#     (positives end at Ln(2^-23) = -15.9; with gumbel max ~15.9 their
#     scores stay far below every threshold ~>2).
#   - score = logw + g   (g = gumbel noise, an input-independent constant
#     reproduced on host exactly as the reference does).
#   - Per-row threshold T with count(score >= T) == k_eff found by a
#     bracketed regula-falsi on log(count) (Newton for 2 warmup steps).
#     Counting is a single fused compare+row-reduce op per tile
#     (tensor_scalar accum / ACT Sign accum).
#   - loss pieces: num = sum(sp*t) - sum(l*t) + sum(sp*sel), den = np + cnt
#     where sp = softplus(l) = Ln(Exp(l)+1) (bce = sp - l*t elementwise).
#   - Host combines the 8 cores' per-row partial sums (pure data parallel).

import os
import subprocess
import sys

import numpy as np
import ml_dtypes

B, C = 8192, 2048
NCORES = 8
BPC = B // NCORES          # 1024 rows per core
MT = BPC // 128            # 8 m-tiles of 128 rows
KT = C // 128              # 16 k-tiles
DIAG = 4096.0
CLMP_HI = float(np.float32(1.0 - 2.0**-23))   # Relu bias
EPS = float(np.float32(2.0**-23))             # Ln bias
LOG_W0 = float(np.log(np.float32(1020.0)))    # analytic T0 = log(1020/k)
NEG_RATIO = 5.0
R_ITERS = int(os.environ.get("NSB_R_ITERS", "10"))
N_NEWTON = 2
GROUPS = (range(0, 4), range(4, 8))
# count-pass engine per tile-slot within a group (v=DVE TS, s=ACT Sign)
COUNT_ENG = os.environ.get("NSB_COUNT_ENG", "vsvs")
LT_ENG = os.environ.get("NSB_LT_ENG", "vector")   # engine for sum(l*t) pass
NP_ENG = os.environ.get("NSB_NP_ENG", "vector")
SPT_ENG = os.environ.get("NSB_SPT_ENG", "vector")

_F32 = None
_BF16 = None
_STATE = {}


def _gumbel_host():
    """Reproduce jax.random.gumbel(jax.random.key(42), (B,C), float32) exactly
    by running CPU-jax in a subprocess (axon boot forces the neuron backend
    in-process).  Falls back to an in-process numpy threefry replica."""
    cache = "/tmp/nsb_gumbel_8192x2048.npy"
    if os.path.exists(cache):
        try:
            g = np.load(cache)
            if g.shape == (B, C) and g.dtype == np.float32:
                return g
        except Exception:
            pass
    code = (
        "import numpy as np, jax\n"
        "g = jax.random.gumbel(jax.random.key(42), (%d, %d), dtype='float32')\n"
        "np.save('%s', np.asarray(g))\n" % (B, C, cache)
    )
    env = dict(os.environ)
    env.pop("TRN_TERMINAL_POOL_IPS", None)
    env["JAX_PLATFORMS"] = "cpu"
    nix_site = None
    try:
        import jax as _jax  # noqa: F401  (resolves under the booted env)
        nix_site = os.path.dirname(os.path.dirname(_jax.__file__))
    except Exception:
        pass
    paths = [p for p in (nix_site,
                         "/root/.axon_site/_ro/trn_rl_repo",
                         "/root/.axon_site/_ro/pypackages") if p]
    env["PYTHONPATH"] = ":".join(paths)
    try:
        subprocess.run([sys.executable, "-c", code], env=env, check=True,
                       capture_output=True, timeout=600)
        g = np.load(cache)
        if g.shape == (B, C) and g.dtype == np.float32:
            return g
    except Exception:
        pass
    return _gumbel_numpy()


def _gumbel_numpy():
    """Numpy replica of jax threefry2x32-based gumbel for key 42 (fallback;
    matches jax's random bits exactly, log() may differ by ~1ulp)."""
    def rotl(x, d):
        return ((x << np.uint32(d)) | (x >> np.uint32(32 - d))).astype(np.uint32)

    def threefry2x32(k0, k1, x0, x1):
        rot = (13, 15, 26, 6, 17, 29, 16, 24)
        ks0, ks1 = k0, k1
        ks2 = (ks0 ^ ks1 ^ np.uint32(0x1BD11BDA)).astype(np.uint32)
        x0 = (x0 + ks0).astype(np.uint32)
        x1 = (x1 + ks1).astype(np.uint32)
        ks = (ks1, ks2, ks0)
        for i in range(5):
            for r in rot[4 * (i % 2):4 * (i % 2) + 4]:
                x0 = (x0 + x1).astype(np.uint32)
                x1 = rotl(x1, r) ^ x0
            x0 = (x0 + ks[i % 3]).astype(np.uint32)
            x1 = (x1 + ks[(i + 1) % 3] + np.uint32(i + 1)).astype(np.uint32)
        return x0, x1

    # jax.random.key(42) -> threefry key (0, 42)
    k0 = np.uint32(0)
    k1 = np.uint32(42)
    n = B * C
    half = (n + 1) // 2
    idx = np.arange(half, dtype=np.uint32)
    x0, x1 = threefry2x32(k0, k1, idx, (idx + np.uint32(half)).astype(np.uint32))
    bits = np.concatenate([x0, x1])[:n]
    tiny = np.float32(np.finfo(np.float32).tiny)
    f = ((bits >> np.uint32(9)) | np.uint32(0x3F800000)).view(np.float32) - np.float32(1.0)
    u = np.maximum(tiny, f * (np.float32(1.0) - tiny) + tiny).astype(np.float32)
    g = (-np.log(-np.log(u).astype(np.float32)).astype(np.float32)).astype(np.float32)
    return g.reshape(B, C)


def _build():
    """Trace + compile the Bass program once per process."""
    if "nc" in _STATE:
        return _STATE["nc"], _STATE["names"]
    import concourse.bacc as bacc
    import concourse.mybir as mybir
    from concourse.tile import TileContext

    global _F32, _BF16
    _F32 = mybir.dt.float32
    _BF16 = mybir.dt.bfloat16
    f32, bf16 = _F32, _BF16
    A = mybir.AluOpType
    AF = mybir.ActivationFunctionType

    nc = bacc.Bacc("TRN2", target_bir_lowering=False, debug=False,
                   num_devices=NCORES)

    l_d = nc.dram_tensor("logits_in", [MT, 128, C], f32, kind="ExternalInput")
    t_d = nc.dram_tensor("targets_in", [MT, 128, C], bf16, kind="ExternalInput")
    g_d = nc.dram_tensor("gumbel_in", [MT, 128, C], f32, kind="ExternalInput")
    pT_d = nc.dram_tensor("posT_in", [C, BPC], bf16, kind="ExternalInput")
    s_d = nc.dram_tensor("simh_in", [C, C], bf16, kind="ExternalInput")

    outs = {}
    for nm in ("np", "cnt", "lt", "spt", "spsel", "tfin"):
        outs[nm] = nc.dram_tensor("out_" + nm, [128, MT], f32,
                                  kind="ExternalOutput")

    def eng(name):
        return {"vector": nc.vector, "scalar": nc.scalar,
                "gpsimd": nc.gpsimd}[name]

    with TileContext(nc) as tc:
        with (
            tc.tile_pool(name="simpool", bufs=1) as simpool,
            tc.tile_pool(name="ptpool", bufs=2) as ptpool,
            tc.tile_pool(name="inpool", bufs=2) as inpool,
            tc.tile_pool(name="gpool", bufs=2) as gpool,
            tc.tile_pool(name="scorepool", bufs=4) as scorepool,
            tc.tile_pool(name="sppool", bufs=4) as sppool,
            tc.tile_pool(name="epool", bufs=2) as epool,
            tc.tile_pool(name="junkpool", bufs=2) as junkpool,
            tc.tile_pool(name="smallpool", bufs=2) as smallpool,
            tc.tile_pool(name="psumpool", bufs=2, space="PSUM") as psumpool,
        ):
            # bias constants for ACT (only 0.0/1.0 are pre-registered)
            c_clmp = simpool.tile([128, 1], f32, tag="c_clmp")
            c_eps = simpool.tile([128, 1], f32, tag="c_eps")
            nc.vector.memset(c_clmp[:], CLMP_HI)
            nc.vector.memset(c_eps[:], EPS)

            # resident bf16 similarity (+4096*I), [128, KT, C]
            sim_sb = simpool.tile([128, KT, C], bf16, tag="sim")
            sview = s_d[:].rearrange("(kt p) n -> p kt n", p=128)
            for kt in range(KT):
                nc.sync.dma_start(sim_sb[:, kt, :], sview[:, kt, :])

            score_t = [None] * MT
            sp_t = [None] * MT

            # per-group [128,4] state
            for gi, grp in enumerate(GROUPS):
                G = len(grp)
                np4 = smallpool.tile([128, G], f32, tag="np4")
                inv4 = smallpool.tile([128, G], f32, tag="inv4")
                ninv4 = smallpool.tile([128, G], f32, tag="ninv4")
                k4 = smallpool.tile([128, G], f32, tag="k4")
                lk4 = smallpool.tile([128, G], f32, tag="lk4")
                T4 = smallpool.tile([128, G], f32, tag="T4")
                nT4 = smallpool.tile([128, G], f32, tag="nT4")
                Tlo4 = smallpool.tile([128, G], f32, tag="Tlo4")
                Thi4 = smallpool.tile([128, G], f32, tag="Thi4")
                lclo4 = smallpool.tile([128, G], f32, tag="lclo4")
                lchi4 = smallpool.tile([128, G], f32, tag="lchi4")
                cnt4 = smallpool.tile([128, G], f32, tag="cnt4")
                tmp1 = smallpool.tile([128, G], f32, tag="tmp1")
                tmp2 = smallpool.tile([128, G], f32, tag="tmp2")
                mge = smallpool.tile([128, G], mybir.dt.uint32, tag="mge")
                mlt = smallpool.tile([128, G], mybir.dt.uint32, tag="mlt")
                npc4 = smallpool.tile([128, G], f32, tag="npc4")
                lt4 = smallpool.tile([128, G], f32, tag="lt4")
                spt4 = smallpool.tile([128, G], f32, tag="spt4")
                spsel4 = smallpool.tile([128, G], f32, tag="spsel4")
                cfin4 = smallpool.tile([128, G], f32, tag="cfin4")

                for j, mt in enumerate(grp):
                    lt_ = inpool.tile([128, C], f32, tag="l")
                    tt_ = inpool.tile([128, C], bf16, tag="t")
                    gt_ = gpool.tile([128, C], f32, tag="g")
                    ptt = ptpool.tile([128, KT, 128], bf16, tag="pt")
                    nc.sync.dma_start(lt_[:], l_d[mt])
                    nc.sync.dma_start(tt_[:], t_d[mt])
                    nc.sync.dma_start(gt_[:], g_d[mt])
                    ptv = pT_d[:, mt * 128:(mt + 1) * 128].rearrange(
                        "(kt p) m -> p kt m", p=128)
                    nc.sync.dma_start(ptt[:], ptv)

                    junk = junkpool.tile([128, C], bf16, tag="junk")
                    # num_pos (sum of targets row)
                    eng(NP_ENG).tensor_scalar(
                        junk[:], tt_[:], 1.0, None, op0=A.mult, op1=A.add,
                        accum_out=np4[:, j:j + 1])

                    # matmul: psum = posT.T @ simh  (accumulate over kt)
                    ps = psumpool.tile([128, C], f32, tag="ps")
                    for kt in range(KT):
                        for ch in range(4):
                            nc.tensor.matmul(
                                ps[:, ch * 512:(ch + 1) * 512],
                                ptt[:, kt, :],
                                sim_sb[:, kt, ch * 512:(ch + 1) * 512],
                                start=(kt == 0), stop=(kt == KT - 1))

                    # per-tile scalars (depend only on np)
                    nc.vector.tensor_scalar(
                        npc4[:, j:j + 1], np4[:, j:j + 1], 1.0, None, op0=A.max)
                    nc.vector.reciprocal(inv4[:, j:j + 1], npc4[:, j:j + 1])
                    nc.vector.tensor_scalar(
                        ninv4[:, j:j + 1], inv4[:, j:j + 1], -1.0, None,
                        op0=A.mult)

                    # y = Relu(psum*(-inv_np) + (1-2^-23));  into score tile
                    sc = scorepool.tile([128, C], f32, tag="score")
                    score_t[mt] = sc
                    nc.scalar.activation(sc[:], ps[:], AF.Relu,
                                         bias=c_clmp[:],
                                         scale=ninv4[:, j:j + 1])
                    # logw = Ln(y + 2^-23) in place
                    nc.scalar.activation(sc[:], sc[:], AF.Ln, bias=c_eps[:])
                    # score += g
                    nc.vector.tensor_add(sc[:], sc[:], gt_[:])

                    # sp = Ln(Exp(l) + 1)  (softplus, one ACT table set)
                    ex = epool.tile([128, C], f32, tag="e")
                    nc.scalar.activation(ex[:], lt_[:], AF.Exp)
                    sp = sppool.tile([128, C], f32, tag="sp")
                    sp_t[mt] = sp
                    nc.scalar.activation(sp[:], ex[:], AF.Ln, bias=1.0)

                    # sum(l*t), sum(sp*t)
                    eng(LT_ENG).scalar_tensor_tensor(
                        junk[:], lt_[:], 1.0, tt_[:], op0=A.mult, op1=A.mult,
                        accum_out=lt4[:, j:j + 1])
                    eng(SPT_ENG).scalar_tensor_tensor(
                        junk[:], sp[:], 1.0, tt_[:], op0=A.mult,
                        op1=A.mult, accum_out=spt4[:, j:j + 1])

                # batched per-group init
                nc.vector.tensor_scalar(npc4[:], np4[:], 1.0, None, op0=A.max)
                nc.vector.tensor_scalar(k4[:], npc4[:], NEG_RATIO, None,
                                        op0=A.mult)
                # nfin = 2048 - np  (reuse tmp1)
                nc.vector.tensor_scalar(tmp1[:], np4[:], -1.0, float(C),
                                        op0=A.mult, op1=A.add)
                nc.vector.tensor_tensor(k4[:], k4[:], tmp1[:], op=A.min)
                nc.scalar.activation(lk4[:], k4[:], AF.Ln)
                nc.scalar.activation(lclo4[:], tmp1[:], AF.Ln)
                nc.vector.tensor_scalar(T4[:], lk4[:], -1.0, LOG_W0,
                                        op0=A.mult, op1=A.add)
                nc.vector.tensor_scalar(nT4[:], T4[:], -1.0, None, op0=A.mult)
                nc.vector.memset(Tlo4[:], -60.0)
                nc.vector.memset(Thi4[:], 20.0)
                nc.vector.memset(lchi4[:], float(np.log(np.float32(0.5))))

                # threshold search
                for it in range(R_ITERS):
                    for j, mt in enumerate(grp):
                        e = COUNT_ENG[j % len(COUNT_ENG)]
                        junk = junkpool.tile([128, C], bf16, tag="junk")
                        if e == "s":
                            nc.scalar.activation(
                                junk[:], score_t[mt][:], AF.Sign,
                                bias=nT4[:, j:j + 1],
                                accum_out=cnt4[:, j:j + 1])
                            # S -> count: (S + 2048) * 0.5
                            nc.vector.tensor_scalar(
                                cnt4[:, j:j + 1], cnt4[:, j:j + 1],
                                float(C), 0.5, op0=A.add, op1=A.mult)
                        else:
                            nc.vector.tensor_scalar(
                                junk[:], score_t[mt][:], T4[:, j:j + 1], None,
                                op0=A.is_ge, op1=A.add,
                                accum_out=cnt4[:, j:j + 1])
                    # batched update
                    nc.vector.tensor_scalar(tmp1[:], cnt4[:], 0.5, None,
                                            op0=A.max)
                    nc.scalar.activation(tmp1[:], tmp1[:], AF.Ln)  # lc
                    nc.vector.tensor_tensor(mge[:], cnt4[:], k4[:], op=A.is_ge)
                    nc.vector.copy_predicated(Tlo4[:], mge[:], T4[:])
                    nc.vector.copy_predicated(lclo4[:], mge[:], tmp1[:])
                    nc.vector.tensor_scalar(mlt[:], mge[:], -1.0, 1.0,
                                            op0=A.mult, op1=A.add)
                    nc.vector.copy_predicated(Thi4[:], mlt[:], T4[:])
                    nc.vector.copy_predicated(lchi4[:], mlt[:], tmp1[:])
                    if it < N_NEWTON:
                        nc.vector.tensor_sub(tmp2[:], tmp1[:], lk4[:])
                        nc.vector.tensor_add(T4[:], T4[:], tmp2[:])
                        nc.vector.tensor_tensor(T4[:], T4[:], Tlo4[:],
                                                op=A.max)
                        nc.vector.tensor_tensor(T4[:], T4[:], Thi4[:],
                                                op=A.min)
                    else:
                        nc.vector.tensor_sub(tmp1[:], lclo4[:], lk4[:])
                        nc.vector.tensor_sub(tmp2[:], lclo4[:], lchi4[:])
                        nc.vector.reciprocal(tmp2[:], tmp2[:])
                        nc.vector.tensor_mul(tmp1[:], tmp1[:], tmp2[:])
                        nc.vector.tensor_sub(tmp2[:], Thi4[:], Tlo4[:])
                        nc.vector.tensor_mul(tmp1[:], tmp1[:], tmp2[:])
                        nc.vector.tensor_add(T4[:], Tlo4[:], tmp1[:])
                    nc.vector.tensor_scalar(nT4[:], T4[:], -1.0, None,
                                            op0=A.mult)

                # final: sel mask, count, sum(sp*sel)
                for j, mt in enumerate(grp):
                    selm = junkpool.tile([128, C], bf16, tag="junk")
                    nc.vector.tensor_scalar(
                        selm[:], score_t[mt][:], T4[:, j:j + 1], None,
                        op0=A.is_ge, op1=A.add, accum_out=cfin4[:, j:j + 1])
                    junk2 = junkpool.tile([128, C], bf16, tag="junk")
                    nc.vector.scalar_tensor_tensor(
                        junk2[:], selm[:], 1.0, sp_t[mt][:], op0=A.mult,
                        op1=A.mult, accum_out=spsel4[:, j:j + 1])

                # write outputs for this group
                lo = gi * len(grp)
                hi = lo + len(grp)
                nc.sync.dma_start(outs["np"][:, lo:hi], np4[:])
                nc.sync.dma_start(outs["cnt"][:, lo:hi], cfin4[:])
                nc.sync.dma_start(outs["lt"][:, lo:hi], lt4[:])
                nc.sync.dma_start(outs["spt"][:, lo:hi], spt4[:])
                nc.sync.dma_start(outs["spsel"][:, lo:hi], spsel4[:])
                nc.sync.dma_start(outs["tfin"][:, lo:hi], T4[:])

    nc.compile()
    names = dict(l="logits_in", t="targets_in", g="gumbel_in", pT="posT_in",
                 s="simh_in")
    _STATE["nc"] = nc
    _STATE["names"] = names
    return nc, names


def _prep_inputs(logits, targets, similarity):
    g = _gumbel_host()
    simh = similarity.astype(ml_dtypes.bfloat16)
    simh[np.arange(C), np.arange(C)] = ml_dtypes.bfloat16(DIAG)
    in_maps = []
    for c in range(NCORES):
        sl = slice(c * BPC, (c + 1) * BPC)
        t_c = targets[sl].astype(ml_dtypes.bfloat16)
        in_maps.append({
            "logits_in": np.ascontiguousarray(
                logits[sl].reshape(MT, 128, C)),
            "targets_in": np.ascontiguousarray(t_c.reshape(MT, 128, C)),
            "gumbel_in": np.ascontiguousarray(g[sl].reshape(MT, 128, C)),
            "posT_in": np.ascontiguousarray(t_c.T),
            "simh_in": simh,
        })
    return in_maps


def kernel(logits, targets, similarity):
    from concourse import bass_utils
    nc, _ = _build()
    in_maps = _prep_inputs(np.asarray(logits, dtype=np.float32),
                           np.asarray(targets, dtype=np.float32),
                           np.asarray(similarity, dtype=np.float32))
    trace = bool(int(os.environ.get("NSB_TRACE", "0")))
    res = bass_utils.run_bass_kernel_spmd(
        nc, in_maps, core_ids=list(range(NCORES)), trace=trace)
    _STATE["last_results"] = res
    num = 0.0
    den = 0.0
    for r in res.results:
        num += (r["out_spt"].astype(np.float64).sum()
                - r["out_lt"].astype(np.float64).sum()
                + r["out_spsel"].astype(np.float64).sum())
        den += (r["out_np"].astype(np.float64).sum()
                + r["out_cnt"].astype(np.float64).sum())
    return np.array(np.float64(num) / np.float64(den), dtype=np.float32)
